# revision 7
# baseline (speedup 1.0000x reference)
"""Trainium2 Bass kernel for the NeRF MLP forward pass.

Strategy: pure data parallel over 8 NeuronCores (32768 rows each), weights
replicated.  Per core:
  - load o/d batch-major (contiguous DMA), 8 super-tiles of 4096 rows
  - angles = x * 2^l (DVE broadcast-AP mult), magic-number range reduction
    to [-pi, pi] (exact round via +/-1.5*2^23), sin on ACT; cos = Sin(pi/2-|r|)
  - PE-transpose embeddings to feature-major [96, cols] (f32r, 1.5 cyc/row)
  - 10 linear layers as f32r matmuls (1 cyc/row @ N=512); b2_3's h-part is
    algebraically fused into b3_0 (W_comb = W8h @ W3a); biases folded into
    weight rows via constant-1 rows in embT where K < 128, ACT bias otherwise
  - sigmoid via tanh; final 0.5*t+0.5 folded into the output mini-transpose
    (selection matrix with 0.5 weights + ones row)
  - PE mini-transposes restore batch-major order for contiguous stores
"""
import os
import numpy as np

N_TOTAL = 262144
N_CORES = 8
NS = N_TOTAL // N_CORES          # rows per core

X = 16                           # rows per partition per super-tile
G = 128 * X                      # super-tile rows = 4096
S = NS // G                      # super-tiles per core = 8
NB = 512                         # matmul column block
L_POS, L_DIR = 10, 4
NFX, NFD = 30, 12                # per-row angle counts (o, d)
NA = NFX + NFD                   # 42
NE = 96                          # embT rows: [sinx 30|cosx 30|pad|ones@60->no]
# embT row layout: 0:30 sinx, 30:60 cosx, 60 ones, 61:64 pad,
#                  64:76 sind, 76:88 cosd, 88 ones, 89:96 pad

MAGIC = 12582912.0               # 1.5 * 2**23
INV2PI = float(1.0 / (2 * np.pi))
_tp = np.float64(2 * np.pi)
C1a = float(np.float32(int(_tp * 2**9) / 2**9))
C1b = float(np.float32(int((_tp - C1a) * 2**21) / 2**21))
C2 = float(np.float32(_tp - C1a - C1b))

GPSIMD_REDUCE = os.environ.get("NERF_GPSIMD", "1") == "1"
TRACE = os.environ.get("NERF_TRACE", "0") == "1"

_cache = {}


def _build():
    import concourse.bacc as bacc
    import concourse.mybir as mybir
    import concourse.tile as tile
    from contextlib import ExitStack

    dt = mybir.dt
    AF = mybir.ActivationFunctionType
    ALU = mybir.AluOpType

    nc = bacc.Bacc("TRN2", target_bir_lowering=False, debug=False)

    o_d = nc.dram_tensor("o", [NS, 3], dt.float32, kind="ExternalInput").ap()
    d_d = nc.dram_tensor("d", [NS, 3], dt.float32, kind="ExternalInput").ap()
    ident_d = nc.dram_tensor("ident", [128, 128], dt.float32, kind="ExternalInput").ap()
    freq_d = nc.dram_tensor("freq", [128, L_POS], dt.float32, kind="ExternalInput").ap()
    sel_d = nc.dram_tensor("sel", [65, 4], dt.float32, kind="ExternalInput").ap()

    WSPEC = [  # name, K, M, partition_base
        ("w1", 61, 128, 0), ("w2", 128, 128, 0), ("w3", 128, 128, 0),
        ("w4", 128, 128, 0), ("w5a", 128, 128, 0), ("w5b", 61, 128, 0),
        ("w6", 128, 128, 0), ("w7", 128, 128, 0), ("wsig", 128, 1, 0),
        ("w9a", 128, 64, 0), ("w9b", 25, 64, 64), ("w10", 64, 3, 0),
    ]
    BSPEC = [("b2", 128), ("b3", 128), ("b4", 128), ("b6", 128), ("b7", 128),
             ("b10h", 3)]
    w_dram = {n: nc.dram_tensor(n, [k, m], dt.float32, kind="ExternalInput").ap()
              for n, k, m, _ in WSPEC}
    b_dram = {n: nc.dram_tensor(n, [m, 1], dt.float32, kind="ExternalInput").ap()
              for n, m in BSPEC}
    bsig_d = nc.dram_tensor("bsig", [1, 1], dt.float32, kind="ExternalInput").ap()

    c_d = nc.dram_tensor("c", [NS, 3], dt.float32, kind="ExternalOutput").ap()
    sig_d = nc.dram_tensor("sigma", [NS], dt.float32, kind="ExternalOutput").ap()

    with tile.TileContext(nc) as tc, ExitStack() as ctx:
        const = ctx.enter_context(tc.tile_pool(name="const", bufs=1))
        sbin = ctx.enter_context(tc.tile_pool(name="sbin", bufs=2))
        sbe = ctx.enter_context(tc.tile_pool(name="sbe", bufs=2))
        sbh = ctx.enter_context(tc.tile_pool(name="sbh", bufs=2))
        sbo = ctx.enter_context(tc.tile_pool(name="sbo", bufs=2))
        ps_t = ctx.enter_context(tc.tile_pool(name="ps_t", bufs=2, space="PSUM"))
        ps_h = ctx.enter_context(tc.tile_pool(name="ps_h", bufs=2, space="PSUM"))
        ps_o = ctx.enter_context(tc.tile_pool(name="ps_o", bufs=1, space="PSUM"))

        red_eng = nc.gpsimd if GPSIMD_REDUCE else nc.vector

        # ---------------- constants ----------------
        ident = const.tile([128, 128], dt.float32)
        nc.sync.dma_start(ident[:], ident_d[:])
        ident_r = const.tile([128, 128], dt.float32r)
        nc.vector.tensor_copy(ident_r[:], ident[:])
        sel = const.tile([65, 4], dt.float32)
        nc.sync.dma_start(sel[:], sel_d[:])
        freq = const.tile([128, L_POS], dt.float32)
        nc.sync.dma_start(freq[:], freq_d[:])
        halfpi = const.tile([128, 1], dt.float32)
        nc.vector.memset(halfpi[:], float(np.pi / 2))
        bsig = const.tile([1, 1], dt.float32)
        nc.sync.dma_start(bsig[:], bsig_d[:])
        ones32 = const.tile([128, 1], dt.float32)
        nc.vector.memset(ones32[:], 1.0)
        ones_r = const.tile([128, 1], dt.float32r)
        nc.vector.tensor_copy(ones_r[:], ones32[:])

        wt = {}
        for n, k, m, base in WSPEC:
            w32 = const.tile([base + k, m], dt.float32, tag=f"w32_{n}")
            nc.sync.dma_start(w32[base:base + k, :], w_dram[n][:])
            wr = const.tile([base + k, m], dt.float32r, tag=f"wr_{n}")
            nc.vector.tensor_copy(wr[base:base + k, :], w32[base:base + k, :])
            wt[n] = wr[base:base + k, :]
        bt = {}
        for n, m in BSPEC:
            b = const.tile([m, 1], dt.float32, tag=f"b_{n}")
            nc.sync.dma_start(b[:], b_dram[n][:])
            bt[n] = b

        # pack: rows 0-2 tanh(c), row 32 ones, row 64 sigma; rest zero
        pack = const.tile([65, G], dt.float32)
        nc.vector.memset(pack[:], 0.0)
        nc.vector.memset(pack[32:33, :], 1.0)

        # ---------------- per super-tile ----------------
        for s in range(S):
            r0 = s * G

            ob = sbin.tile([128, 3 * X], dt.float32, tag="ob")
            nc.sync.dma_start(ob[:], o_d[r0:r0 + G, :].rearrange("(p x) c -> p (x c)", p=128))
            db = sbin.tile([128, 3 * X], dt.float32, tag="db")
            nc.sync.dma_start(db[:], d_d[r0:r0 + G, :].rearrange("(p x) c -> p (x c)", p=128))

            # angles
            ang = sbe.tile([128, NA * X], dt.float32, tag="ang")
            ang4 = ang[:].rearrange("p (x j) -> p x j", j=NA)
            in0 = ob[:].rearrange("p (x c) -> p x c", c=3).unsqueeze(3) \
                       .broadcast_to([128, X, 3, L_POS])
            in1 = freq[:].unsqueeze(1).unsqueeze(1).broadcast_to([128, X, 3, L_POS])
            nc.vector.tensor_tensor(
                ang4[:, :, 0:NFX].rearrange("p x (c l) -> p x c l", l=L_POS),
                in0, in1, op=ALU.mult)
            in0 = db[:].rearrange("p (x c) -> p x c", c=3).unsqueeze(3) \
                       .broadcast_to([128, X, 3, L_DIR])
            in1 = freq[:, 0:L_DIR].unsqueeze(1).unsqueeze(1) \
                       .broadcast_to([128, X, 3, L_DIR])
            nc.vector.tensor_tensor(
                ang4[:, :, NFX:NA].rearrange("p x (c l) -> p x c l", l=L_DIR),
                in0, in1, op=ALU.mult)

            # range reduction (on red_eng): k = round(ang/2pi); r = ang - k*2pi
            kt = sbe.tile([128, NA * X], dt.float32, tag="kt")
            red_eng.tensor_scalar(kt[:], ang[:], INV2PI, MAGIC, op0=ALU.mult, op1=ALU.add)
            red_eng.tensor_scalar(kt[:], kt[:], MAGIC, None, op0=ALU.subtract)
            red = sbe.tile([128, NA * X], dt.float32, tag="red")
            u = sbe.tile([128, NA * X], dt.float32, tag="u")
            red_eng.tensor_scalar(u[:], kt[:], C1a, None, op0=ALU.mult)
            nc.vector.tensor_tensor(red[:], ang[:], u[:], op=ALU.subtract)
            red_eng.tensor_scalar(u[:], kt[:], C1b, None, op0=ALU.mult)
            nc.vector.tensor_tensor(red[:], red[:], u[:], op=ALU.subtract)
            red_eng.tensor_scalar(u[:], kt[:], C2, None, op0=ALU.mult)
            nc.vector.tensor_tensor(red[:], red[:], u[:], op=ALU.subtract)
            red_eng.tensor_scalar(red[:], red[:], -float(np.pi), float(np.pi),
                                  op0=ALU.max, op1=ALU.min)
            # |r| for cos
            aq = sbe.tile([128, NA * X], dt.float32, tag="kt")
            red_eng.tensor_scalar(aq[:], red[:], -1.0, None, op0=ALU.mult)
            nc.vector.tensor_tensor(aq[:], aq[:], red[:], op=ALU.max)

            # embeddings, batch-major f32r
            emb = sbe.tile([128, NE * X], dt.float32r, tag="emb")
            embv = emb[:].rearrange("p (x e) -> p x e", e=NE)
            redv = red[:].rearrange("p (x j) -> p x j", j=NA)
            aqv = aq[:].rearrange("p (x j) -> p x j", j=NA)
            nc.scalar.activation(embv[:, :, 0:NFX], redv[:, :, 0:NFX], AF.Sin)
            nc.scalar.activation(embv[:, :, NFX:60], aqv[:, :, 0:NFX], AF.Sin,
                                 scale=-1.0, bias=halfpi[:])
            nc.scalar.activation(embv[:, :, 64:76], redv[:, :, NFX:NA], AF.Sin)
            nc.scalar.activation(embv[:, :, 76:88], aqv[:, :, NFX:NA], AF.Sin,
                                 scale=-1.0, bias=halfpi[:])
            ones_b = ones_r[:].broadcast_to([128, X])
            nc.vector.tensor_copy(embv[:, :, 60:61].squeeze(2), ones_b)  # bias ones (x)
            nc.vector.tensor_copy(embv[:, :, 88:89].squeeze(2), ones_b)  # bias ones (d)

            # transpose to embT [NE, G] f32r
            embT = sbh.tile([NE, G], dt.float32r, tag="embT")
            for h in range(X // 4):
                pT = ps_t.tile([NE, NB], dt.float32r, tag="embt_ps")
                for rr in range(4):
                    r = h * 4 + rr
                    nc.tensor.transpose(pT[:, rr * 128:(rr + 1) * 128],
                                        emb[:, r * NE:(r + 1) * NE], ident_r[:])
                nc.vector.tensor_copy(embT[:, h * NB:(h + 1) * NB], pT[:])

            # ---- layers ----
            def mlp_layer(dst, srcs, act_engine, bias=None, M=128):
                """dst [M, G] f32r; srcs = [(wtile, rhs_fn(col0))]"""
                for hh in range(G // 1024):
                    p = ps_h.tile([M, 1024], dt.float32, tag="hpsum")
                    for i, (w, rhs_fn) in enumerate(srcs):
                        for cb in range(2):
                            c0 = hh * 1024 + cb * NB
                            nc.tensor.matmul(p[:, cb * NB:(cb + 1) * NB], w[:],
                                             rhs_fn(c0), start=(i == 0),
                                             stop=(i == len(srcs) - 1))
                    cols = slice(hh * 1024, (hh + 1) * 1024)
                    if act_engine == "act":
                        nc.scalar.activation(dst[:, cols], p[:], AF.Relu, bias=bias[:])
                    else:
                        nc.vector.tensor_scalar(dst[:, cols], p[:], 0.0, None,
                                                op0=ALU.max)

            h1 = sbh.tile([128, G], dt.float32r, tag="hA")
            mlp_layer(h1, [(wt["w1"], lambda c0: embT[0:61, c0:c0 + NB])], "dve")
            h2 = sbh.tile([128, G], dt.float32r, tag="hB")
            mlp_layer(h2, [(wt["w2"], lambda c0: h1[:, c0:c0 + NB])], "act", bt["b2"])
            h3 = sbh.tile([128, G], dt.float32r, tag="hA")
            mlp_layer(h3, [(wt["w3"], lambda c0: h2[:, c0:c0 + NB])], "act", bt["b3"])
            h4 = sbh.tile([128, G], dt.float32r, tag="hB")
            mlp_layer(h4, [(wt["w4"], lambda c0: h3[:, c0:c0 + NB])], "act", bt["b4"])
            h5 = sbh.tile([128, G], dt.float32r, tag="hA")
            mlp_layer(h5, [(wt["w5a"], lambda c0: h4[:, c0:c0 + NB]),
                           (wt["w5b"], lambda c0: embT[0:61, c0:c0 + NB])], "dve")
            h6 = sbh.tile([128, G], dt.float32r, tag="hB")
            mlp_layer(h6, [(wt["w6"], lambda c0: h5[:, c0:c0 + NB])], "act", bt["b6"])
            t7 = sbh.tile([128, G], dt.float32r, tag="hA")
            mlp_layer(t7, [(wt["w7"], lambda c0: h6[:, c0:c0 + NB])], "act", bt["b7"])
            h9 = sbh.tile([64, G], dt.float32r, tag="hB")
            mlp_layer(h9, [(wt["w9a"], lambda c0: t7[:, c0:c0 + NB]),
                           (wt["w9b"], lambda c0: embT[64:89, c0:c0 + NB])], "dve", M=64)

            # c pre-sigmoid (tanh) and sigma
            for hh in range(G // 1024):
                cols = slice(hh * 1024, (hh + 1) * 1024)
                pc = ps_h.tile([3, 1024], dt.float32, tag="hpsum")
                for cb in range(2):
                    c0 = hh * 1024 + cb * NB
                    nc.tensor.matmul(pc[:, cb * NB:(cb + 1) * NB], wt["w10"][:],
                                     h9[:, c0:c0 + NB], start=True, stop=True)
                nc.scalar.activation(pack[0:3, cols], pc[:], AF.Tanh,
                                     bias=bt["b10h"][:], scale=0.5)
                psg = ps_h.tile([1, 1024], dt.float32, tag="hpsum")
                for cb in range(2):
                    c0 = hh * 1024 + cb * NB
                    nc.tensor.matmul(psg[:, cb * NB:(cb + 1) * NB], wt["wsig"][:],
                                     t7[:, c0:c0 + NB], start=True, stop=True)
                nc.vector.tensor_scalar(pack[64:65, cols], psg[:], bsig[:], 0.0,
                                        op0=ALU.add, op1=ALU.max)

            # output mini-transposes: [65, 128] x sel[65, 4] -> [128, 4]
            # col0 = sigma; cols 1-3 = 0.5*tanh + 0.5 (sigmoid finish)
            p_all = ps_o.tile([128, 4 * X], dt.float32, tag="pout")
            for r in range(X):
                nc.tensor.matmul(p_all[:, r * 4:(r + 1) * 4],
                                 pack[:, r * 128:(r + 1) * 128], sel[:],
                                 start=True, stop=True)
            outp = sbo.tile([128, 4 * X], dt.float32, tag="outp")
            nc.vector.tensor_copy(outp[:], p_all[:])

            ov = outp[:].rearrange("p (x e) -> p x e", e=4)
            nc.sync.dma_start(c_d[r0:r0 + G, :].rearrange("(p x) c -> p x c", p=128),
                              ov[:, :, 1:4])
            nc.sync.dma_start(sig_d[r0:r0 + G].rearrange("(p x) -> p x", p=128),
                              ov[:, :, 0])

    nc.compile()
    return nc


def _prep_inputs(o, d, params):
    o = np.ascontiguousarray(np.asarray(o, dtype=np.float32))
    d = np.ascontiguousarray(np.asarray(d, dtype=np.float32))
    p = {k: (np.asarray(W, np.float64), np.asarray(b, np.float64))
         for k, (W, b) in params.items()}

    perm60 = np.array([(r // 10) * 20 + 2 * (r % 10) if r < 30
                       else ((r - 30) // 10) * 20 + 2 * ((r - 30) % 10) + 1
                       for r in range(60)])
    perm24 = np.array([(r // 4) * 8 + 2 * (r % 4) if r < 12
                       else ((r - 12) // 4) * 8 + 2 * ((r - 12) % 4) + 1
                       for r in range(24)])

    W8, b8 = p['b2_3']
    W3, b3 = p['b3_0']
    W_comb = W8[:, :128] @ W3[:128]
    bias_comb = b8[:128] @ W3[:128] + b3

    def aug(Wp, bias):      # append bias row (ones-row trick)
        return np.concatenate([Wp, bias[None, :]], axis=0)

    f32 = lambda a: np.ascontiguousarray(a.astype(np.float32))
    shared = {
        "ident": np.eye(128, dtype=np.float32),
        "freq": np.broadcast_to((2.0 ** np.arange(L_POS)).astype(np.float32),
                                (128, L_POS)).copy(),
        "w1": f32(aug(p['b1_0'][0][perm60], p['b1_0'][1])),
        "w2": f32(p['b1_1'][0]), "b2": f32(p['b1_1'][1][:, None]),
        "w3": f32(p['b1_2'][0]), "b3": f32(p['b1_2'][1][:, None]),
        "w4": f32(p['b1_3'][0]), "b4": f32(p['b1_3'][1][:, None]),
        "w5a": f32(p['b2_0'][0][:128]),
        "w5b": f32(aug(p['b2_0'][0][128:][perm60], p['b2_0'][1])),
        "w6": f32(p['b2_1'][0]), "b6": f32(p['b2_1'][1][:, None]),
        "w7": f32(p['b2_2'][0]), "b7": f32(p['b2_2'][1][:, None]),
        "wsig": f32(W8[:, 128:129]),
        "bsig": f32(b8[128:129][:, None]),
        "w9a": f32(W_comb),
        "w9b": f32(aug(W3[128:][perm24], bias_comb)),
        "w10": f32(p['b4_0'][0]),
        "b10h": f32(p['b4_0'][1][:, None] * 0.5),
    }
    sel = np.zeros((65, 4), dtype=np.float32)
    sel[64, 0] = 1.0                    # sigma passthrough
    for ch in range(3):
        sel[ch, 1 + ch] = 0.5           # 0.5 * tanh
        sel[32, 1 + ch] = 0.5           # + 0.5 (ones row)
    shared["sel"] = sel
    return o, d, shared


def kernel(o, d, params):
    from concourse.bass_utils import run_bass_kernel_spmd

    o, d, shared = _prep_inputs(o, d, params)

    if "nc" not in _cache:
        _cache["nc"] = _build()
    nc = _cache["nc"]

    in_maps = []
    for i in range(N_CORES):
        m = dict(shared)
        m["o"] = o[i * NS:(i + 1) * NS]
        m["d"] = d[i * NS:(i + 1) * NS]
        in_maps.append(m)

    out = run_bass_kernel_spmd(nc, in_maps, list(range(N_CORES)), trace=TRACE)
    if TRACE and out.exec_time_ns is not None:
        print(f"HW exec time: {out.exec_time_ns} ns")
        _cache["exec_time_ns"] = out.exec_time_ns
    res = out.results
    c = np.concatenate([res[i]["c"] for i in range(N_CORES)], axis=0)
    sigma = np.concatenate([res[i]["sigma"] for i in range(N_CORES)], axis=0)
    return c, sigma
